# revision 2
# baseline (speedup 1.0000x reference)
"""Trainium2 Bass kernel for nn_RRE_GNN_raw (GNN message passing).

Strategy: sort edges by destination node (obj) on the host, shard NODES
across the 8 cores (each core owns 49 node-tiles of 128 nodes and all
edges pointing into them -> no collectives). Per node-tile, the device
gathers per-edge rows (hidden[sub], rela_embed[rel], hq[r_idx]) via
indirect DMA, computes the GRU gate + attention in feature-major f16
matmuls, and reduces the softmax-weighted segment sums with scaled
one-hot matmuls accumulated in PSUM.
"""
import sys

sys.path.insert(0, '/opt/trn_rl_repo')

import json
import numpy as np

import concourse.bass as bass
import concourse.tile as tile
from concourse import mybir
from concourse.bass_utils import run_bass_kernel_spmd
from concourse.vector_clock import ScopedClock
import bass_rust

# ---------------------------------------------------------------- constants
P = 128            # partitions / tile edge
D = 128            # feature dim
A = 128            # attention dim
N_NODE = 50000
E_EDGE = 600000
NQ = 1024
NRE = 401
NCORES = 8
T_TILES = 49       # node tiles per core
NODES_PER_CORE = T_TILES * P          # 6272
N_PAD = NCORES * NODES_PER_CORE       # 50176
MACRO = 4          # chunks fused per PSUM gate group (N = MACRO*128 <= 512)

f16 = mybir.dt.float16
f32 = mybir.dt.float32
i32 = mybir.dt.int32

AF = mybir.ActivationFunctionType
ALU = mybir.AluOpType


# ------------------------------------------------- harness compatibility fixes
class _TC(tile.TileContext):
    """TileContext whose kernel-tail drain emits one wait per instruction
    (the walrus build here rejects instructions with >1 inline sync wait)."""

    def _drain_and_barrier(self, tick_clock, wait_clock):
        nc = self.nc
        probe = nc.sync.nop(nofuse=True)
        wait_clock.add_sem_waits(probe.ins,
                                 ScopedClock({None: tick_clock.global_clock}))
        waits = list(probe.ins.sync_info.on_wait)
        probe.ins.sync_info = bass_rust.SyncInfo(on_wait=[], on_update=[])
        name2sem = {s.name: s for s in self.sems.allocated().values()}
        for w in waits:
            nc.sync.wait_ge(name2sem[w.ant_name], w.wait_value)
        nc.sync.drain()
        nc.all_engine_barrier()
        popped = nc._tile_sem_poison_stack.pop()
        assert popped is self._sem_poison
        nc.clear_and_free_semaphores(list(self.sems.allocated().values()))
        nc.all_engine_barrier()


def _split_bir_waits(bir_json: bytes) -> bytes:
    """Hoist all-but-one sync wait of any instruction onto standalone
    EventSemaphore ops placed just before it on the same engine queue."""
    d = json.loads(bir_json)
    changed = False
    for func in d.get("functions", []):
        for blk in func.get("blocks", []):
            out = []
            for inst in blk["instructions"]:
                si = inst.get("sync_info")
                waits = si.get("on_wait", []) if si else []
                if len(waits) > 1:
                    for k, w in enumerate(waits[:-1]):
                        out.append({
                            "name": f"{inst['name']}-hw{k}",
                            "opcode": "EventSemaphore",
                            "engine": inst["engine"],
                            "ins": [], "outs": [],
                            "sync_info": {"on_update": [], "on_wait": [w]},
                        })
                    si["on_wait"] = waits[-1:]
                    changed = True
                out.append(inst)
            blk["instructions"] = out
    if not changed:
        return bir_json
    return json.dumps(d).encode()


_hook_installed = False


def _install_wait_splitter():
    global _hook_installed
    if _hook_installed:
        return
    import concourse.bass2jax as bass2jax
    orig = bass2jax.compile_bir_kernel

    def patched(bir_json, tmpdir, neff_name="file.neff"):
        return orig(_split_bir_waits(bir_json), tmpdir, neff_name=neff_name)

    bass2jax.compile_bir_kernel = patched
    _hook_installed = True


# ---------------------------------------------------------------- host prep
def _host_prep(hidden, rela_embed, q_rel, edges):
    """Sort/shard/pad on the host. Returns per-core metadata arrays and the
    static per-tile chunk counts (shared by all cores -> one SPMD program)."""
    r_idx = edges[:, 0].astype(np.int64)
    rel = edges[:, 2].astype(np.int64)
    sub = edges[:, 4].astype(np.int64)
    obj = edges[:, 5].astype(np.int64)

    order = np.argsort(obj, kind="stable")
    obj_s = obj[order]
    sub_s = sub[order]
    rel_s = rel[order]
    rix_s = r_idx[order]

    # node_group: last write in ORIGINAL edge order (matches reference)
    node_group = np.zeros(N_PAD, np.int64)
    node_group[obj] = r_idx

    counts = np.bincount(obj_s, minlength=N_PAD)
    starts = np.zeros(N_PAD + 1, np.int64)
    np.cumsum(counts, out=starts[1:])

    n_gtiles = NCORES * T_TILES
    gc = np.zeros(n_gtiles, np.int64)   # edges per global node-tile
    for g in range(n_gtiles):
        gc[g] = starts[min((g + 1) * P, N_PAD)] - starts[g * P]
    chunks = (gc + P - 1) // P
    # per tile-index max over cores (same program on every core)
    C_list = [max(1, int(chunks[t::T_TILES].max())) for t in range(T_TILES)]
    col_off = np.zeros(T_TILES + 1, np.int64)
    np.cumsum(C_list, out=col_off[1:])
    CT = int(col_off[-1])

    off_sub = np.zeros((NCORES, P, CT), np.int32)
    off_rel = np.zeros((NCORES, P, CT), np.int32)
    off_rix = np.zeros((NCORES, P, CT), np.int32)
    obj_f = np.full((NCORES, P, CT), -1.0, np.float32)

    for core in range(NCORES):
        for t in range(T_TILES):
            g = core * T_TILES + t
            lo = starts[g * P]
            hi = starts[min((g + 1) * P, N_PAD)]
            L = int(hi - lo)
            Ct = C_list[t]
            slot = np.arange(L)
            pp = slot % P
            cc = col_off[t] + slot // P
            off_sub[core, pp, cc] = sub_s[lo:hi]
            off_rel[core, pp, cc] = rel_s[lo:hi]
            off_rix[core, pp, cc] = rix_s[lo:hi]
            obj_f[core, pp, cc] = (obj_s[lo:hi] - g * P).astype(np.float32)

    ng_off = node_group.reshape(NCORES, T_TILES, P).transpose(0, 2, 1) \
                       .astype(np.int32).copy()    # [core, P, T]

    return dict(
        C_list=C_list, col_off=col_off, CT=CT,
        off_sub=off_sub, off_rel=off_rel, off_rix=off_rix,
        obj_f=obj_f, ng_off=ng_off,
    )


# ------------------------------------------------------------ device program
def _build_program(C_list, col_off, CT):
    nc = bass.Bass()
    dp = nc.declare_dram_parameter

    hid16 = dp("hid16", [N_NODE, D], f16, isOutput=False)
    rela16 = dp("rela16", [NRE, D], f16, isOutput=False)
    hq16 = dp("hq16", [NQ, D], f16, isOutput=False)
    hq32 = dp("hq32", [NQ, D], f32, isOutput=False)

    wz_t = dp("wz_t", [D, D], f16, isOutput=False)
    wz_b = dp("wz_b", [D, D], f16, isOutput=False)
    uz = dp("uz", [D, D], f16, isOutput=False)
    wr_t = dp("wr_t", [D, D], f16, isOutput=False)
    wr_b = dp("wr_b", [D, D], f16, isOutput=False)
    ur = dp("ur", [D, D], f16, isOutput=False)
    wh_t = dp("wh_t", [D, D], f16, isOutput=False)
    wh_b = dp("wh_b", [D, D], f16, isOutput=False)
    uh = dp("uh", [D, D], f16, isOutput=False)
    ws = dp("ws", [D, A], f16, isOutput=False)
    wqr = dp("wqr", [D, A], f16, isOutput=False)
    walpha = dp("walpha", [A, 1], f16, isOutput=False)
    wh_out = dp("wh_out", [D, D], f16, isOutput=False)
    bz = dp("bz", [D, 1], f32, isOutput=False)
    br = dp("br", [D, 1], f32, isOutput=False)
    bh = dp("bh", [D, 1], f32, isOutput=False)
    bqr = dp("bqr", [A, 1], f32, isOutput=False)
    balpha = dp("balpha", [P, 1], f32, isOutput=False)
    iota_d = dp("iota", [P, P], f32, isOutput=False)

    off_sub_d = dp("off_sub", [P, CT], i32, isOutput=False)
    off_rel_d = dp("off_rel", [P, CT], i32, isOutput=False)
    off_rix_d = dp("off_rix", [P, CT], i32, isOutput=False)
    obj_f_d = dp("obj_f", [P, CT], f32, isOutput=False)
    ng_off_d = dp("ng_off", [P, T_TILES], i32, isOutput=False)

    out_ht = dp("out_ht", [P, T_TILES * P], f32, isOutput=True)
    out_hnqr = dp("out_hnqr", [T_TILES * P, D], f32, isOutput=True)

    Cmax = max(C_list)

    from contextlib import ExitStack
    with _TC(nc) as tc, ExitStack() as ctx:
        const = ctx.enter_context(tc.tile_pool(name="const", bufs=1))
        meta = ctx.enter_context(tc.tile_pool(name="meta", bufs=1))
        gat = ctx.enter_context(tc.tile_pool(name="gat", bufs=2))
        trn = ctx.enter_context(tc.tile_pool(name="trn", bufs=2))
        ck = ctx.enter_context(tc.tile_pool(name="ck", bufs=3))
        fin = ctx.enter_context(tc.tile_pool(name="fin", bufs=2))
        psg = ctx.enter_context(tc.tile_pool(name="psg", bufs=1, space="PSUM"))
        psc = ctx.enter_context(tc.tile_pool(name="psc", bufs=1, space="PSUM"))
        psa = ctx.enter_context(tc.tile_pool(name="psa", bufs=2, space="PSUM"))
        psf = ctx.enter_context(tc.tile_pool(name="psf", bufs=1, space="PSUM"))

        def load(pool, dram_t, shape, dt, tag):
            t = pool.tile(shape, dt, tag=tag)
            nc.sync.dma_start(t[:], dram_t[:])
            return t

        wz_t_s = load(const, wz_t, [D, D], f16, "wz_t")
        wz_b_s = load(const, wz_b, [D, D], f16, "wz_b")
        uz_s = load(const, uz, [D, D], f16, "uz")
        wr_t_s = load(const, wr_t, [D, D], f16, "wr_t")
        wr_b_s = load(const, wr_b, [D, D], f16, "wr_b")
        ur_s = load(const, ur, [D, D], f16, "ur")
        wh_t_s = load(const, wh_t, [D, D], f16, "wh_t")
        wh_b_s = load(const, wh_b, [D, D], f16, "wh_b")
        uh_s = load(const, uh, [D, D], f16, "uh")
        ws_s = load(const, ws, [D, A], f16, "ws")
        wqr_s = load(const, wqr, [D, A], f16, "wqr")
        walpha_s = load(const, walpha, [A, 1], f16, "walpha")
        whout_s = load(const, wh_out, [D, D], f16, "whout")
        bz_s = load(const, bz, [D, 1], f32, "bz")
        br_s = load(const, br, [D, 1], f32, "br")
        bh_s = load(const, bh, [D, 1], f32, "bh")
        bqr_s = load(const, bqr, [A, 1], f32, "bqr")
        balpha_s = load(const, balpha, [P, 1], f32, "balpha")
        iota_s = load(const, iota_d, [P, P], f32, "iota")
        ones_s = const.tile([P, 1], f16, tag="ones")
        nc.vector.memset(ones_s[:], 1.0)

        off_sub_s = load(meta, off_sub_d, [P, CT], i32, "off_sub")
        off_rel_s = load(meta, off_rel_d, [P, CT], i32, "off_rel")
        off_rix_s = load(meta, off_rix_d, [P, CT], i32, "off_rix")
        obj_f_s = load(meta, obj_f_d, [P, CT], f32, "obj_f")
        ng_off_s = load(meta, ng_off_d, [P, T_TILES], i32, "ng_off")

        mm = nc.tensor.matmul
        act = nc.scalar.activation

        for t in range(T_TILES):
            Ct = C_list[t]
            co = int(col_off[t])
            Et = Ct * P

            hs_raw = gat.tile([P, Cmax * P], f16, tag="hs_raw")
            hr_raw = gat.tile([P, Cmax * P], f16, tag="hr_raw")
            hqr_raw = gat.tile([P, Cmax * P], f16, tag="hqr_raw")
            for c in range(Ct):
                sl = slice(c * P, (c + 1) * P)
                nc.gpsimd.indirect_dma_start(
                    out=hs_raw[:, sl], out_offset=None, in_=hid16[:],
                    in_offset=bass.IndirectOffsetOnAxis(
                        ap=off_sub_s[:, co + c:co + c + 1], axis=0))
                nc.gpsimd.indirect_dma_start(
                    out=hr_raw[:, sl], out_offset=None, in_=rela16[:],
                    in_offset=bass.IndirectOffsetOnAxis(
                        ap=off_rel_s[:, co + c:co + c + 1], axis=0))
                nc.gpsimd.indirect_dma_start(
                    out=hqr_raw[:, sl], out_offset=None, in_=hq16[:],
                    in_offset=bass.IndirectOffsetOnAxis(
                        ap=off_rix_s[:, co + c:co + c + 1], axis=0))

            hsT = trn.tile([P, Cmax * P], f16, tag="hsT")
            hrT = trn.tile([P, Cmax * P], f16, tag="hrT")
            hqrT = trn.tile([P, Cmax * P], f16, tag="hqrT")
            nc.sync.dma_start_transpose(
                out=hsT[:, :Et].rearrange("p (k d) -> p k d", k=Ct),
                in_=hs_raw[:, :Et])
            nc.sync.dma_start_transpose(
                out=hrT[:, :Et].rearrange("p (k d) -> p k d", k=Ct),
                in_=hr_raw[:, :Et])
            nc.sync.dma_start_transpose(
                out=hqrT[:, :Et].rearrange("p (k d) -> p k d", k=Ct),
                in_=hqr_raw[:, :Et])

            agg = psa.tile([P, 132], f32, tag="agg")

            for m0 in range(0, Ct, MACRO):
                mc = min(MACRO, Ct - m0)
                Em = mc * P
                sl = slice(m0 * P, m0 * P + Em)

                zp = psg.tile([P, MACRO * P], f32, tag="zp")
                rp = psg.tile([P, MACRO * P], f32, tag="rp")
                hp = psg.tile([P, MACRO * P], f32, tag="hp")
                apre = psg.tile([P, MACRO * P], f32, tag="apre")

                mm(zp[:, :Em], lhsT=wz_t_s[:], rhs=hrT[:, sl],
                   start=True, stop=False)
                mm(zp[:, :Em], lhsT=wz_b_s[:], rhs=hqrT[:, sl],
                   start=False, stop=False)
                mm(zp[:, :Em], lhsT=uz_s[:], rhs=hsT[:, sl],
                   start=False, stop=True)

                mm(rp[:, :Em], lhsT=wr_t_s[:], rhs=hrT[:, sl],
                   start=True, stop=False)
                mm(rp[:, :Em], lhsT=wr_b_s[:], rhs=hqrT[:, sl],
                   start=False, stop=False)
                mm(rp[:, :Em], lhsT=ur_s[:], rhs=hsT[:, sl],
                   start=False, stop=True)

                mm(hp[:, :Em], lhsT=wh_t_s[:], rhs=hrT[:, sl],
                   start=True, stop=False)
                mm(hp[:, :Em], lhsT=wh_b_s[:], rhs=hqrT[:, sl],
                   start=False, stop=False)

                z_sb = ck.tile([P, MACRO * P], f16, tag="z")
                act(z_sb[:, :Em], zp[:, :Em], AF.Sigmoid, bias=bz_s[:, :1])
                r_sb = ck.tile([P, MACRO * P], f16, tag="r")
                act(r_sb[:, :Em], rp[:, :Em], AF.Sigmoid, bias=br_s[:, :1])

                rh = ck.tile([P, MACRO * P], f16, tag="rh")
                nc.vector.tensor_tensor(out=rh[:, :Em], in0=r_sb[:, :Em],
                                        in1=hsT[:, sl], op=ALU.mult)
                mm(hp[:, :Em], lhsT=uh_s[:], rhs=rh[:, :Em],
                   start=False, stop=True)
                ht = ck.tile([P, MACRO * P], f16, tag="ht")
                act(ht[:, :Em], hp[:, :Em], AF.Tanh, bias=bh_s[:, :1])

                # message^T = hsT + z*(ht - hsT)
                dd = ck.tile([P, MACRO * P], f16, tag="dd")
                nc.vector.tensor_tensor(out=dd[:, :Em], in0=ht[:, :Em],
                                        in1=hsT[:, sl], op=ALU.subtract)
                zd = ck.tile([P, MACRO * P], f16, tag="zd")
                nc.vector.tensor_tensor(out=zd[:, :Em], in0=z_sb[:, :Em],
                                        in1=dd[:, :Em], op=ALU.mult)
                msgT = ck.tile([P, MACRO * P], f16, tag="msgT")
                nc.vector.tensor_tensor(out=msgT[:, :Em], in0=zd[:, :Em],
                                        in1=hsT[:, sl], op=ALU.add)

                mm(apre[:, :Em], lhsT=ws_s[:], rhs=msgT[:, :Em],
                   start=True, stop=False)
                mm(apre[:, :Em], lhsT=wqr_s[:], rhs=hqrT[:, sl],
                   start=False, stop=True)
                relu_sb = ck.tile([P, MACRO * P], f16, tag="relu")
                act(relu_sb[:, :Em], apre[:, :Em], AF.Relu, bias=bqr_s[:, :1])

                msgE = ck.tile([P, MACRO * P], f16, tag="msgE")
                nc.sync.dma_start_transpose(
                    out=msgE[:, :Em].rearrange("p (k d) -> p k d", k=mc),
                    in_=msgT[:, :Em])

                acol = psc.tile([P, MACRO], f32, tag="acol")
                for cl in range(mc):
                    c = m0 + cl
                    csl = slice(cl * P, (cl + 1) * P)
                    mm(acol[:, cl:cl + 1], lhsT=relu_sb[:, csl],
                       rhs=walpha_s[:], start=True, stop=True)
                    expc = ck.tile([P, 1], f32, tag="expc")
                    act(expc[:, :1], acol[:, cl:cl + 1], AF.Exp,
                        bias=balpha_s[:, :1])
                    pw = ck.tile([P, P], f16, tag="pw")
                    nc.vector.tensor_scalar(
                        out=pw[:], in0=iota_s[:],
                        scalar1=obj_f_s[:, co + c:co + c + 1],
                        scalar2=expc[:, :1],
                        op0=ALU.is_equal, op1=ALU.mult)
                    # start=True clears the whole PSUM bank, so only the
                    # first matmul of the bank may use it; the sumexp
                    # column relies on that bank clear (a start=False mm
                    # on cleared has_written bits writes fresh).
                    mm(agg[:, 0:P], lhsT=pw[:], rhs=msgE[:, csl],
                       start=(c == 0), stop=(c == Ct - 1),
                       skip_group_check=True)
                    mm(agg[:, P:P + 1], lhsT=pw[:], rhs=ones_s[:],
                       start=False, stop=(c == Ct - 1),
                       skip_group_check=True)

            recip = fin.tile([P, 1], f32, tag="recip")
            nc.vector.reciprocal(recip[:], agg[:, P:P + 1])
            magg = fin.tile([P, P], f16, tag="magg")
            nc.vector.tensor_scalar(out=magg[:], in0=agg[:, 0:P],
                                    scalar1=recip[:, :1], scalar2=None,
                                    op0=ALU.mult)
            maggT = fin.tile([P, P], f16, tag="maggT")
            nc.sync.dma_start_transpose(out=maggT[:], in_=magg[:])
            hf = psf.tile([P, P], f32, tag="hf")
            mm(hf[:], lhsT=whout_s[:], rhs=maggT[:], start=True, stop=True)
            hnew = fin.tile([P, P], f32, tag="hnew")
            act(hnew[:], hf[:], AF.Relu)
            nc.sync.dma_start(out_ht[:, t * P:(t + 1) * P], hnew[:])

            hnq = fin.tile([P, D], f32, tag="hnq")
            nc.gpsimd.indirect_dma_start(
                out=hnq[:], out_offset=None, in_=hq32[:],
                in_offset=bass.IndirectOffsetOnAxis(
                    ap=ng_off_s[:, t:t + 1], axis=0))
            nc.sync.dma_start(out_hnqr[t * P:(t + 1) * P, :], hnq[:])

    return nc


# ----------------------------------------------------------------- kernel()
def kernel(hidden, rela_embed, Wz, Uz, bz, Wr_g, Ur, br, Whh, Uh, bh,
           Ws_attn, Wqr_attn, b_qr, w_alpha, b_alpha, W_h,
           q_rel, edges, n_node):
    _install_wait_splitter()

    hidden = np.asarray(hidden, np.float32)
    rela_embed = np.asarray(rela_embed, np.float32)
    edges = np.asarray(edges)
    q_rel = np.asarray(q_rel)

    meta = _host_prep(hidden, rela_embed, q_rel, edges)
    C_list, col_off, CT = meta["C_list"], meta["col_off"], meta["CT"]

    hq = rela_embed[np.asarray(q_rel, np.int64)]          # [NQ, D] f32

    nc = _build_program(C_list, col_off, CT)

    common = {
        "hid16": hidden.astype(np.float16),
        "rela16": rela_embed.astype(np.float16),
        "hq16": hq.astype(np.float16),
        "hq32": hq.astype(np.float32),
        "wz_t": np.asarray(Wz[:D], np.float16),
        "wz_b": np.asarray(Wz[D:], np.float16),
        "uz": np.asarray(Uz, np.float16),
        "wr_t": np.asarray(Wr_g[:D], np.float16),
        "wr_b": np.asarray(Wr_g[D:], np.float16),
        "ur": np.asarray(Ur, np.float16),
        "wh_t": np.asarray(Whh[:D], np.float16),
        "wh_b": np.asarray(Whh[D:], np.float16),
        "uh": np.asarray(Uh, np.float16),
        "ws": np.asarray(Ws_attn, np.float16),
        "wqr": np.asarray(Wqr_attn, np.float16),
        "walpha": np.asarray(w_alpha, np.float16).reshape(A, 1),
        "wh_out": np.asarray(W_h, np.float16),
        "bz": np.asarray(bz, np.float32).reshape(D, 1),
        "br": np.asarray(br, np.float32).reshape(D, 1),
        "bh": np.asarray(bh, np.float32).reshape(D, 1),
        "bqr": np.asarray(b_qr, np.float32).reshape(A, 1),
        "balpha": np.full((P, 1), float(np.asarray(b_alpha).reshape(-1)[0]),
                          np.float32),
        "iota": np.broadcast_to(np.arange(P, dtype=np.float32),
                                (P, P)).copy(),
    }
    in_maps = []
    for core in range(NCORES):
        m = dict(common)
        m["off_sub"] = meta["off_sub"][core]
        m["off_rel"] = meta["off_rel"][core]
        m["off_rix"] = meta["off_rix"][core]
        m["obj_f"] = meta["obj_f"][core]
        m["ng_off"] = meta["ng_off"][core]
        in_maps.append(m)

    res = run_bass_kernel_spmd(nc, in_maps, list(range(NCORES))).results

    hidden_new = np.empty((N_PAD, D), np.float32)
    h_n_qr = np.empty((N_PAD, D), np.float32)
    for core in range(NCORES):
        lo = core * NODES_PER_CORE
        hi = lo + NODES_PER_CORE
        hidden_new[lo:hi] = res[core]["out_ht"].T
        h_n_qr[lo:hi] = res[core]["out_hnqr"]

    return hidden_new[:N_NODE], h_n_qr[:N_NODE]


# revision 3
# speedup vs baseline: 1.0784x; 1.0784x over previous
"""Trainium2 Bass kernel for nn_RRE_GNN_raw (GNN message passing).

Strategy: sort edges by destination node (obj) on the host, shard NODES
across the 8 cores (each core owns 49 node-tiles of 128 nodes and all
edges pointing into them -> no collectives). Per node-tile, the device
gathers per-edge rows (hidden[sub], rela_embed[rel], hq[r_idx]) via
indirect DMA, computes the GRU gate + attention in feature-major f16
matmuls, and reduces the softmax-weighted segment sums with scaled
one-hot matmuls accumulated in PSUM.
"""
import sys

sys.path.insert(0, '/opt/trn_rl_repo')

import json
import numpy as np

import concourse.bass as bass
import concourse.tile as tile
from concourse import mybir
from concourse.bass_utils import run_bass_kernel_spmd
from concourse.vector_clock import ScopedClock
import bass_rust

# ---------------------------------------------------------------- constants
P = 128            # partitions / tile edge
D = 128            # feature dim
A = 128            # attention dim
N_NODE = 50000
E_EDGE = 600000
NQ = 1024
NRE = 401
NCORES = 8
T_TILES = 49       # node tiles per core
NODES_PER_CORE = T_TILES * P          # 6272
N_PAD = NCORES * NODES_PER_CORE       # 50176
MACRO = 4          # chunks fused per PSUM gate group (N = MACRO*128 <= 512)
ABLATE_NO_GATHER = False
ABLATE_GATHER_ONLY = False

f16 = mybir.dt.float16
f32 = mybir.dt.float32
i32 = mybir.dt.int32

AF = mybir.ActivationFunctionType
ALU = mybir.AluOpType


# ------------------------------------------------- harness compatibility fixes
class _TC(tile.TileContext):
    """TileContext whose kernel-tail drain emits one wait per instruction
    (the walrus build here rejects instructions with >1 inline sync wait)."""

    def _drain_and_barrier(self, tick_clock, wait_clock):
        nc = self.nc
        probe = nc.sync.nop(nofuse=True)
        wait_clock.add_sem_waits(probe.ins,
                                 ScopedClock({None: tick_clock.global_clock}))
        waits = list(probe.ins.sync_info.on_wait)
        probe.ins.sync_info = bass_rust.SyncInfo(on_wait=[], on_update=[])
        name2sem = {s.name: s for s in self.sems.allocated().values()}
        for w in waits:
            nc.sync.wait_ge(name2sem[w.ant_name], w.wait_value)
        nc.sync.drain()
        nc.all_engine_barrier()
        popped = nc._tile_sem_poison_stack.pop()
        assert popped is self._sem_poison
        nc.clear_and_free_semaphores(list(self.sems.allocated().values()))
        nc.all_engine_barrier()


def _split_bir_waits(bir_json: bytes) -> bytes:
    """Hoist all-but-one sync wait of any instruction onto standalone
    EventSemaphore ops placed just before it on the same engine queue."""
    d = json.loads(bir_json)
    changed = False
    for func in d.get("functions", []):
        for blk in func.get("blocks", []):
            out = []
            for inst in blk["instructions"]:
                si = inst.get("sync_info")
                waits = si.get("on_wait", []) if si else []
                if len(waits) > 1:
                    for k, w in enumerate(waits[:-1]):
                        out.append({
                            "name": f"{inst['name']}-hw{k}",
                            "opcode": "EventSemaphore",
                            "engine": inst["engine"],
                            "ins": [], "outs": [],
                            "sync_info": {"on_update": [], "on_wait": [w]},
                        })
                    si["on_wait"] = waits[-1:]
                    changed = True
                out.append(inst)
            blk["instructions"] = out
    if not changed:
        return bir_json
    return json.dumps(d).encode()


_hook_installed = False


def _install_wait_splitter():
    global _hook_installed
    if _hook_installed:
        return
    import concourse.bass2jax as bass2jax
    orig = bass2jax.compile_bir_kernel

    def patched(bir_json, tmpdir, neff_name="file.neff"):
        return orig(_split_bir_waits(bir_json), tmpdir, neff_name=neff_name)

    bass2jax.compile_bir_kernel = patched
    _hook_installed = True


# ---------------------------------------------------------------- host prep
def _host_prep(hidden, rela_embed, q_rel, edges):
    """Sort/shard/pad on the host. Returns per-core metadata arrays and the
    static per-tile chunk counts (shared by all cores -> one SPMD program)."""
    r_idx = edges[:, 0].astype(np.int64)
    rel = edges[:, 2].astype(np.int64)
    sub = edges[:, 4].astype(np.int64)
    obj = edges[:, 5].astype(np.int64)

    order = np.argsort(obj, kind="stable")
    obj_s = obj[order]
    sub_s = sub[order]
    rel_s = rel[order]
    rix_s = r_idx[order]

    # node_group: last write in ORIGINAL edge order (matches reference)
    node_group = np.zeros(N_PAD, np.int64)
    node_group[obj] = r_idx

    counts = np.bincount(obj_s, minlength=N_PAD)
    starts = np.zeros(N_PAD + 1, np.int64)
    np.cumsum(counts, out=starts[1:])

    n_gtiles = NCORES * T_TILES
    gc = np.zeros(n_gtiles, np.int64)   # edges per global node-tile
    for g in range(n_gtiles):
        gc[g] = starts[min((g + 1) * P, N_PAD)] - starts[g * P]
    chunks = (gc + P - 1) // P
    # per tile-index max over cores (same program on every core)
    C_list = [max(1, int(chunks[t::T_TILES].max())) for t in range(T_TILES)]
    col_off = np.zeros(T_TILES + 1, np.int64)
    np.cumsum(C_list, out=col_off[1:])
    CT = int(col_off[-1])

    off_sub = np.zeros((NCORES, P, CT), np.int32)
    off_rel = np.zeros((NCORES, P, CT), np.int32)
    off_rix = np.zeros((NCORES, P, CT), np.int32)
    obj_f = np.full((NCORES, P, CT), -1.0, np.float32)

    for core in range(NCORES):
        for t in range(T_TILES):
            g = core * T_TILES + t
            lo = starts[g * P]
            hi = starts[min((g + 1) * P, N_PAD)]
            L = int(hi - lo)
            Ct = C_list[t]
            slot = np.arange(L)
            pp = slot % P
            cc = col_off[t] + slot // P
            off_sub[core, pp, cc] = sub_s[lo:hi]
            off_rel[core, pp, cc] = rel_s[lo:hi]
            off_rix[core, pp, cc] = rix_s[lo:hi]
            obj_f[core, pp, cc] = (obj_s[lo:hi] - g * P).astype(np.float32)

    ng_off = node_group.reshape(NCORES, T_TILES, P).transpose(0, 2, 1) \
                       .astype(np.int32).copy()    # [core, P, T]

    return dict(
        C_list=C_list, col_off=col_off, CT=CT,
        off_sub=off_sub, off_rel=off_rel, off_rix=off_rix,
        obj_f=obj_f, ng_off=ng_off,
    )


# ------------------------------------------------------------ device program
def _build_program(C_list, col_off, CT):
    nc = bass.Bass()
    dp = nc.declare_dram_parameter

    hid16 = dp("hid16", [N_NODE, D], f16, isOutput=False)
    rela16 = dp("rela16", [NRE, D], f16, isOutput=False)
    hq16 = dp("hq16", [NQ, D], f16, isOutput=False)
    hq32 = dp("hq32", [NQ, D], f32, isOutput=False)

    wz_t = dp("wz_t", [D, D], f16, isOutput=False)
    wz_b = dp("wz_b", [D, D], f16, isOutput=False)
    uz = dp("uz", [D, D], f16, isOutput=False)
    wr_t = dp("wr_t", [D, D], f16, isOutput=False)
    wr_b = dp("wr_b", [D, D], f16, isOutput=False)
    ur = dp("ur", [D, D], f16, isOutput=False)
    wh_t = dp("wh_t", [D, D], f16, isOutput=False)
    wh_b = dp("wh_b", [D, D], f16, isOutput=False)
    uh = dp("uh", [D, D], f16, isOutput=False)
    ws = dp("ws", [D, A], f16, isOutput=False)
    wqr = dp("wqr", [D, A], f16, isOutput=False)
    walpha = dp("walpha", [A, 1], f16, isOutput=False)
    wh_out = dp("wh_out", [D, D], f16, isOutput=False)
    bz = dp("bz", [D, 1], f32, isOutput=False)
    br = dp("br", [D, 1], f32, isOutput=False)
    bh = dp("bh", [D, 1], f32, isOutput=False)
    bqr = dp("bqr", [A, 1], f32, isOutput=False)
    balpha = dp("balpha", [P, 1], f32, isOutput=False)
    iota_d = dp("iota", [P, P], f32, isOutput=False)

    off_sub_d = dp("off_sub", [P, CT], i32, isOutput=False)
    off_rel_d = dp("off_rel", [P, CT], i32, isOutput=False)
    off_rix_d = dp("off_rix", [P, CT], i32, isOutput=False)
    obj_f_d = dp("obj_f", [P, CT], f32, isOutput=False)
    ng_off_d = dp("ng_off", [P, T_TILES], i32, isOutput=False)

    out_ht = dp("out_ht", [P, T_TILES * P], f32, isOutput=True)
    out_hnqr = dp("out_hnqr", [T_TILES * P, D], f32, isOutput=True)

    Cmax = max(C_list)

    from contextlib import ExitStack
    with _TC(nc) as tc, ExitStack() as ctx:
        const = ctx.enter_context(tc.tile_pool(name="const", bufs=1))
        meta = ctx.enter_context(tc.tile_pool(name="meta", bufs=1))
        gat = ctx.enter_context(tc.tile_pool(name="gat", bufs=2))
        trn = ctx.enter_context(tc.tile_pool(name="trn", bufs=2))
        ck = ctx.enter_context(tc.tile_pool(name="ck", bufs=3))
        fin = ctx.enter_context(tc.tile_pool(name="fin", bufs=2))
        psg = ctx.enter_context(tc.tile_pool(name="psg", bufs=1, space="PSUM"))
        psc = ctx.enter_context(tc.tile_pool(name="psc", bufs=1, space="PSUM"))
        psa = ctx.enter_context(tc.tile_pool(name="psa", bufs=2, space="PSUM"))
        psf = ctx.enter_context(tc.tile_pool(name="psf", bufs=1, space="PSUM"))

        def load(pool, dram_t, shape, dt, tag):
            t = pool.tile(shape, dt, tag=tag)
            nc.sync.dma_start(t[:], dram_t[:])
            return t

        wz_t_s = load(const, wz_t, [D, D], f16, "wz_t")
        wz_b_s = load(const, wz_b, [D, D], f16, "wz_b")
        uz_s = load(const, uz, [D, D], f16, "uz")
        wr_t_s = load(const, wr_t, [D, D], f16, "wr_t")
        wr_b_s = load(const, wr_b, [D, D], f16, "wr_b")
        ur_s = load(const, ur, [D, D], f16, "ur")
        wh_t_s = load(const, wh_t, [D, D], f16, "wh_t")
        wh_b_s = load(const, wh_b, [D, D], f16, "wh_b")
        uh_s = load(const, uh, [D, D], f16, "uh")
        ws_s = load(const, ws, [D, A], f16, "ws")
        wqr_s = load(const, wqr, [D, A], f16, "wqr")
        walpha_s = load(const, walpha, [A, 1], f16, "walpha")
        whout_s = load(const, wh_out, [D, D], f16, "whout")
        bz_s = load(const, bz, [D, 1], f32, "bz")
        br_s = load(const, br, [D, 1], f32, "br")
        bh_s = load(const, bh, [D, 1], f32, "bh")
        bqr_s = load(const, bqr, [A, 1], f32, "bqr")
        balpha_s = load(const, balpha, [P, 1], f32, "balpha")
        iota_s = load(const, iota_d, [P, P], f32, "iota")
        ones_s = const.tile([P, 1], f16, tag="ones")
        nc.vector.memset(ones_s[:], 1.0)

        off_sub_s = load(meta, off_sub_d, [P, CT], i32, "off_sub")
        off_rel_s = load(meta, off_rel_d, [P, CT], i32, "off_rel")
        off_rix_s = load(meta, off_rix_d, [P, CT], i32, "off_rix")
        obj_f_s = load(meta, obj_f_d, [P, CT], f32, "obj_f")
        ng_off_s = load(meta, ng_off_d, [P, T_TILES], i32, "ng_off")

        mm = nc.tensor.matmul
        act = nc.scalar.activation

        for t in range(T_TILES):
            Ct = C_list[t]
            co = int(col_off[t])
            Et = Ct * P

            hs_raw = gat.tile([P, Cmax * P], f16, tag="hs_raw")
            hr_raw = gat.tile([P, Cmax * P], f16, tag="hr_raw")
            hqr_raw = gat.tile([P, Cmax * P], f16, tag="hqr_raw")
            for c in range(Ct if not ABLATE_NO_GATHER else 0):
                sl = slice(c * P, (c + 1) * P)
                nc.gpsimd.indirect_dma_start(
                    out=hs_raw[:, sl], out_offset=None, in_=hid16[:],
                    in_offset=bass.IndirectOffsetOnAxis(
                        ap=off_sub_s[:, co + c:co + c + 1], axis=0))
                nc.gpsimd.indirect_dma_start(
                    out=hr_raw[:, sl], out_offset=None, in_=rela16[:],
                    in_offset=bass.IndirectOffsetOnAxis(
                        ap=off_rel_s[:, co + c:co + c + 1], axis=0))
                nc.gpsimd.indirect_dma_start(
                    out=hqr_raw[:, sl], out_offset=None, in_=hq16[:],
                    in_offset=bass.IndirectOffsetOnAxis(
                        ap=off_rix_s[:, co + c:co + c + 1], axis=0))

            if ABLATE_GATHER_ONLY:
                continue
            hsT = trn.tile([P, Cmax * P], f16, tag="hsT")
            hrT = trn.tile([P, Cmax * P], f16, tag="hrT")
            hqrT = trn.tile([P, Cmax * P], f16, tag="hqrT")
            nc.sync.dma_start_transpose(
                out=hsT[:, :Et].rearrange("p (k d) -> p k d", k=Ct),
                in_=hs_raw[:, :Et])
            nc.sync.dma_start_transpose(
                out=hrT[:, :Et].rearrange("p (k d) -> p k d", k=Ct),
                in_=hr_raw[:, :Et])
            nc.sync.dma_start_transpose(
                out=hqrT[:, :Et].rearrange("p (k d) -> p k d", k=Ct),
                in_=hqr_raw[:, :Et])

            agg = psa.tile([P, 132], f32, tag="agg")

            for m0 in range(0, Ct, MACRO):
                mc = min(MACRO, Ct - m0)
                Em = mc * P
                sl = slice(m0 * P, m0 * P + Em)

                zp = psg.tile([P, MACRO * P], f32, tag="zp")
                rp = psg.tile([P, MACRO * P], f32, tag="rp")
                hp = psg.tile([P, MACRO * P], f32, tag="hp")
                apre = psg.tile([P, MACRO * P], f32, tag="apre")

                mm(zp[:, :Em], lhsT=wz_t_s[:], rhs=hrT[:, sl],
                   start=True, stop=False)
                mm(zp[:, :Em], lhsT=wz_b_s[:], rhs=hqrT[:, sl],
                   start=False, stop=False)
                mm(zp[:, :Em], lhsT=uz_s[:], rhs=hsT[:, sl],
                   start=False, stop=True)

                mm(rp[:, :Em], lhsT=wr_t_s[:], rhs=hrT[:, sl],
                   start=True, stop=False)
                mm(rp[:, :Em], lhsT=wr_b_s[:], rhs=hqrT[:, sl],
                   start=False, stop=False)
                mm(rp[:, :Em], lhsT=ur_s[:], rhs=hsT[:, sl],
                   start=False, stop=True)

                mm(hp[:, :Em], lhsT=wh_t_s[:], rhs=hrT[:, sl],
                   start=True, stop=False)
                mm(hp[:, :Em], lhsT=wh_b_s[:], rhs=hqrT[:, sl],
                   start=False, stop=False)

                z_sb = ck.tile([P, MACRO * P], f16, tag="z")
                act(z_sb[:, :Em], zp[:, :Em], AF.Sigmoid, bias=bz_s[:, :1])
                r_sb = ck.tile([P, MACRO * P], f16, tag="r")
                act(r_sb[:, :Em], rp[:, :Em], AF.Sigmoid, bias=br_s[:, :1])

                rh = ck.tile([P, MACRO * P], f16, tag="rh")
                nc.vector.tensor_tensor(out=rh[:, :Em], in0=r_sb[:, :Em],
                                        in1=hsT[:, sl], op=ALU.mult)
                mm(hp[:, :Em], lhsT=uh_s[:], rhs=rh[:, :Em],
                   start=False, stop=True)
                ht = ck.tile([P, MACRO * P], f16, tag="ht")
                act(ht[:, :Em], hp[:, :Em], AF.Tanh, bias=bh_s[:, :1])

                # message^T = hsT + z*(ht - hsT)
                dd = ck.tile([P, MACRO * P], f16, tag="dd")
                nc.vector.tensor_tensor(out=dd[:, :Em], in0=ht[:, :Em],
                                        in1=hsT[:, sl], op=ALU.subtract)
                zd = ck.tile([P, MACRO * P], f16, tag="zd")
                nc.vector.tensor_tensor(out=zd[:, :Em], in0=z_sb[:, :Em],
                                        in1=dd[:, :Em], op=ALU.mult)
                msgT = ck.tile([P, MACRO * P], f16, tag="msgT")
                nc.vector.tensor_tensor(out=msgT[:, :Em], in0=zd[:, :Em],
                                        in1=hsT[:, sl], op=ALU.add)

                mm(apre[:, :Em], lhsT=ws_s[:], rhs=msgT[:, :Em],
                   start=True, stop=False)
                mm(apre[:, :Em], lhsT=wqr_s[:], rhs=hqrT[:, sl],
                   start=False, stop=True)
                relu_sb = ck.tile([P, MACRO * P], f16, tag="relu")
                act(relu_sb[:, :Em], apre[:, :Em], AF.Relu, bias=bqr_s[:, :1])

                msgE = ck.tile([P, MACRO * P], f16, tag="msgE")
                nc.sync.dma_start_transpose(
                    out=msgE[:, :Em].rearrange("p (k d) -> p k d", k=mc),
                    in_=msgT[:, :Em])

                acol = psc.tile([P, MACRO], f32, tag="acol")
                for cl in range(mc):
                    c = m0 + cl
                    csl = slice(cl * P, (cl + 1) * P)
                    mm(acol[:, cl:cl + 1], lhsT=relu_sb[:, csl],
                       rhs=walpha_s[:], start=True, stop=True)
                    expc = ck.tile([P, 1], f32, tag="expc")
                    act(expc[:, :1], acol[:, cl:cl + 1], AF.Exp,
                        bias=balpha_s[:, :1])
                    pw = ck.tile([P, P], f16, tag="pw")
                    nc.vector.tensor_scalar(
                        out=pw[:], in0=iota_s[:],
                        scalar1=obj_f_s[:, co + c:co + c + 1],
                        scalar2=expc[:, :1],
                        op0=ALU.is_equal, op1=ALU.mult)
                    # start=True clears the whole PSUM bank, so only the
                    # first matmul of the bank may use it; the sumexp
                    # column relies on that bank clear (a start=False mm
                    # on cleared has_written bits writes fresh).
                    mm(agg[:, 0:P], lhsT=pw[:], rhs=msgE[:, csl],
                       start=(c == 0), stop=(c == Ct - 1),
                       skip_group_check=True)
                    mm(agg[:, P:P + 1], lhsT=pw[:], rhs=ones_s[:],
                       start=False, stop=(c == Ct - 1),
                       skip_group_check=True)

            recip = fin.tile([P, 1], f32, tag="recip")
            nc.vector.reciprocal(recip[:], agg[:, P:P + 1])
            magg = fin.tile([P, P], f16, tag="magg")
            nc.vector.tensor_scalar(out=magg[:], in0=agg[:, 0:P],
                                    scalar1=recip[:, :1], scalar2=None,
                                    op0=ALU.mult)
            maggT = fin.tile([P, P], f16, tag="maggT")
            nc.sync.dma_start_transpose(out=maggT[:], in_=magg[:])
            hf = psf.tile([P, P], f32, tag="hf")
            mm(hf[:], lhsT=whout_s[:], rhs=maggT[:], start=True, stop=True)
            hnew = fin.tile([P, P], f32, tag="hnew")
            act(hnew[:], hf[:], AF.Relu)
            nc.sync.dma_start(out_ht[:, t * P:(t + 1) * P], hnew[:])

            hnq = fin.tile([P, D], f32, tag="hnq")
            nc.gpsimd.indirect_dma_start(
                out=hnq[:], out_offset=None, in_=hq32[:],
                in_offset=bass.IndirectOffsetOnAxis(
                    ap=ng_off_s[:, t:t + 1], axis=0))
            nc.sync.dma_start(out_hnqr[t * P:(t + 1) * P, :], hnq[:])

    return nc


# ----------------------------------------------------------------- kernel()
def kernel(hidden, rela_embed, Wz, Uz, bz, Wr_g, Ur, br, Whh, Uh, bh,
           Ws_attn, Wqr_attn, b_qr, w_alpha, b_alpha, W_h,
           q_rel, edges, n_node):
    _install_wait_splitter()

    hidden = np.asarray(hidden, np.float32)
    rela_embed = np.asarray(rela_embed, np.float32)
    edges = np.asarray(edges)
    q_rel = np.asarray(q_rel)

    meta = _host_prep(hidden, rela_embed, q_rel, edges)
    C_list, col_off, CT = meta["C_list"], meta["col_off"], meta["CT"]

    hq = rela_embed[np.asarray(q_rel, np.int64)]          # [NQ, D] f32

    nc = _build_program(C_list, col_off, CT)

    common = {
        "hid16": hidden.astype(np.float16),
        "rela16": rela_embed.astype(np.float16),
        "hq16": hq.astype(np.float16),
        "hq32": hq.astype(np.float32),
        "wz_t": np.asarray(Wz[:D], np.float16),
        "wz_b": np.asarray(Wz[D:], np.float16),
        "uz": np.asarray(Uz, np.float16),
        "wr_t": np.asarray(Wr_g[:D], np.float16),
        "wr_b": np.asarray(Wr_g[D:], np.float16),
        "ur": np.asarray(Ur, np.float16),
        "wh_t": np.asarray(Whh[:D], np.float16),
        "wh_b": np.asarray(Whh[D:], np.float16),
        "uh": np.asarray(Uh, np.float16),
        "ws": np.asarray(Ws_attn, np.float16),
        "wqr": np.asarray(Wqr_attn, np.float16),
        "walpha": np.asarray(w_alpha, np.float16).reshape(A, 1),
        "wh_out": np.asarray(W_h, np.float16),
        "bz": np.asarray(bz, np.float32).reshape(D, 1),
        "br": np.asarray(br, np.float32).reshape(D, 1),
        "bh": np.asarray(bh, np.float32).reshape(D, 1),
        "bqr": np.asarray(b_qr, np.float32).reshape(A, 1),
        "balpha": np.full((P, 1), float(np.asarray(b_alpha).reshape(-1)[0]),
                          np.float32),
        "iota": np.broadcast_to(np.arange(P, dtype=np.float32),
                                (P, P)).copy(),
    }
    in_maps = []
    for core in range(NCORES):
        m = dict(common)
        m["off_sub"] = meta["off_sub"][core]
        m["off_rel"] = meta["off_rel"][core]
        m["off_rix"] = meta["off_rix"][core]
        m["obj_f"] = meta["obj_f"][core]
        m["ng_off"] = meta["ng_off"][core]
        in_maps.append(m)

    res = run_bass_kernel_spmd(nc, in_maps, list(range(NCORES))).results

    hidden_new = np.empty((N_PAD, D), np.float32)
    h_n_qr = np.empty((N_PAD, D), np.float32)
    for core in range(NCORES):
        lo = core * NODES_PER_CORE
        hi = lo + NODES_PER_CORE
        hidden_new[lo:hi] = res[core]["out_ht"].T
        h_n_qr[lo:hi] = res[core]["out_hnqr"]

    return hidden_new[:N_NODE], h_n_qr[:N_NODE]


# revision 4
# speedup vs baseline: 1.3444x; 1.2467x over previous
"""Trainium2 Bass kernel for nn_RRE_GNN_raw (GNN message passing).

Strategy: sort edges by destination node (obj) on the host, shard NODES
across the 8 cores (each core owns 49 node-tiles of 128 nodes and all
edges pointing into them -> no collectives). Per node-tile, the device
gathers per-edge rows (hidden[sub], rela_embed[rel], hq[r_idx]) via
indirect DMA, computes the GRU gate + attention in feature-major f16
matmuls, and reduces the softmax-weighted segment sums with scaled
one-hot matmuls accumulated in PSUM.
"""
import sys

sys.path.insert(0, '/opt/trn_rl_repo')

import json
import numpy as np

import concourse.bass as bass
import concourse.tile as tile
from concourse import mybir
from concourse.bass_utils import run_bass_kernel_spmd
from concourse.vector_clock import ScopedClock
import bass_rust

# ---------------------------------------------------------------- constants
P = 128            # partitions / tile edge
D = 128            # feature dim
A = 128            # attention dim
N_NODE = 50000
E_EDGE = 600000
NQ = 1024
NRE = 401
NCORES = 8
T_TILES = 49       # node tiles per core
NODES_PER_CORE = T_TILES * P          # 6272
N_PAD = NCORES * NODES_PER_CORE       # 50176
MACRO = 4          # chunks fused per PSUM gate group (N = MACRO*128 <= 512)
ABLATE_NO_GATHER = False
ABLATE_GATHER_ONLY = False

f16 = mybir.dt.float16
f32 = mybir.dt.float32
i32 = mybir.dt.int32

AF = mybir.ActivationFunctionType
ALU = mybir.AluOpType


# ------------------------------------------------- harness compatibility fixes
class _TC(tile.TileContext):
    """TileContext whose kernel-tail drain emits one wait per instruction
    (the walrus build here rejects instructions with >1 inline sync wait)."""

    def _drain_and_barrier(self, tick_clock, wait_clock):
        nc = self.nc
        probe = nc.sync.nop(nofuse=True)
        wait_clock.add_sem_waits(probe.ins,
                                 ScopedClock({None: tick_clock.global_clock}))
        waits = list(probe.ins.sync_info.on_wait)
        probe.ins.sync_info = bass_rust.SyncInfo(on_wait=[], on_update=[])
        name2sem = {s.name: s for s in self.sems.allocated().values()}
        for w in waits:
            nc.sync.wait_ge(name2sem[w.ant_name], w.wait_value)
        nc.sync.drain()
        nc.all_engine_barrier()
        popped = nc._tile_sem_poison_stack.pop()
        assert popped is self._sem_poison
        nc.clear_and_free_semaphores(list(self.sems.allocated().values()))
        nc.all_engine_barrier()


def _split_bir_waits(bir_json: bytes) -> bytes:
    """Hoist all-but-one sync wait of any instruction onto standalone
    EventSemaphore ops placed just before it on the same engine queue."""
    d = json.loads(bir_json)
    changed = False
    for func in d.get("functions", []):
        for blk in func.get("blocks", []):
            out = []
            for inst in blk["instructions"]:
                si = inst.get("sync_info")
                waits = si.get("on_wait", []) if si else []
                if len(waits) > 1:
                    for k, w in enumerate(waits[:-1]):
                        out.append({
                            "name": f"{inst['name']}-hw{k}",
                            "opcode": "EventSemaphore",
                            "engine": inst["engine"],
                            "ins": [], "outs": [],
                            "sync_info": {"on_update": [], "on_wait": [w]},
                        })
                    si["on_wait"] = waits[-1:]
                    changed = True
                out.append(inst)
            blk["instructions"] = out
    if not changed:
        return bir_json
    return json.dumps(d).encode()


_hook_installed = False


def _install_wait_splitter():
    global _hook_installed
    if _hook_installed:
        return
    import concourse.bass2jax as bass2jax
    orig = bass2jax.compile_bir_kernel

    def patched(bir_json, tmpdir, neff_name="file.neff"):
        return orig(_split_bir_waits(bir_json), tmpdir, neff_name=neff_name)

    bass2jax.compile_bir_kernel = patched
    _hook_installed = True


# ---------------------------------------------------------------- host prep
def _host_prep(hidden, rela_embed, q_rel, edges):
    """Sort/shard/pad on the host. Returns per-core metadata arrays and the
    static per-tile chunk counts (shared by all cores -> one SPMD program)."""
    r_idx = edges[:, 0].astype(np.int64)
    rel = edges[:, 2].astype(np.int64)
    sub = edges[:, 4].astype(np.int64)
    obj = edges[:, 5].astype(np.int64)

    order = np.argsort(obj, kind="stable")
    obj_s = obj[order]
    sub_s = sub[order]
    rel_s = rel[order]
    rix_s = r_idx[order]

    # node_group: last write in ORIGINAL edge order (matches reference)
    node_group = np.zeros(N_PAD, np.int64)
    node_group[obj] = r_idx

    counts = np.bincount(obj_s, minlength=N_PAD)
    starts = np.zeros(N_PAD + 1, np.int64)
    np.cumsum(counts, out=starts[1:])

    n_gtiles = NCORES * T_TILES
    gc = np.zeros(n_gtiles, np.int64)   # edges per global node-tile
    for g in range(n_gtiles):
        gc[g] = starts[min((g + 1) * P, N_PAD)] - starts[g * P]
    chunks = (gc + P - 1) // P
    # per tile-index max over cores (same program on every core)
    C_list = [max(1, int(chunks[t::T_TILES].max())) for t in range(T_TILES)]
    col_off = np.zeros(T_TILES + 1, np.int64)
    np.cumsum(C_list, out=col_off[1:])
    CT = int(col_off[-1])

    off_sub = np.zeros((NCORES, P, CT), np.int32)
    off_rel = np.zeros((NCORES, P, CT), np.int32)
    off_rix = np.zeros((NCORES, P, CT), np.int32)
    obj_f = np.full((NCORES, P, CT), -1.0, np.float32)

    for core in range(NCORES):
        for t in range(T_TILES):
            g = core * T_TILES + t
            lo = starts[g * P]
            hi = starts[min((g + 1) * P, N_PAD)]
            L = int(hi - lo)
            Ct = C_list[t]
            slot = np.arange(L)
            pp = slot % P
            cc = col_off[t] + slot // P
            off_sub[core, pp, cc] = sub_s[lo:hi]
            off_rel[core, pp, cc] = rel_s[lo:hi]
            off_rix[core, pp, cc] = rix_s[lo:hi]
            obj_f[core, pp, cc] = (obj_s[lo:hi] - g * P).astype(np.float32)

    ng_off = node_group.reshape(NCORES, T_TILES, P).transpose(0, 2, 1) \
                       .astype(np.int32).copy()    # [core, P, T]

    return dict(
        C_list=C_list, col_off=col_off, CT=CT,
        off_sub=off_sub, off_rel=off_rel, off_rix=off_rix,
        obj_f=obj_f, ng_off=ng_off,
    )


# ------------------------------------------------------------ device program
def _build_program(C_list, col_off, CT):
    nc = bass.Bass()
    dp = nc.declare_dram_parameter

    hid16 = dp("hid16", [N_NODE, D], f16, isOutput=False)
    rela16 = dp("rela16", [NRE, D], f16, isOutput=False)
    hq16 = dp("hq16", [NQ, D], f16, isOutput=False)
    hq32 = dp("hq32", [NQ, D], f32, isOutput=False)

    wz_t = dp("wz_t", [D, D], f16, isOutput=False)
    wz_b = dp("wz_b", [D, D], f16, isOutput=False)
    uz = dp("uz", [D, D], f16, isOutput=False)
    wr_t = dp("wr_t", [D, D], f16, isOutput=False)
    wr_b = dp("wr_b", [D, D], f16, isOutput=False)
    ur = dp("ur", [D, D], f16, isOutput=False)
    wh_t = dp("wh_t", [D, D], f16, isOutput=False)
    wh_b = dp("wh_b", [D, D], f16, isOutput=False)
    uh = dp("uh", [D, D], f16, isOutput=False)
    ws = dp("ws", [D, A], f16, isOutput=False)
    wqr = dp("wqr", [D, A], f16, isOutput=False)
    walpha = dp("walpha", [A, 1], f16, isOutput=False)
    wh_out = dp("wh_out", [D, D], f16, isOutput=False)
    bz = dp("bz", [D, 1], f32, isOutput=False)
    br = dp("br", [D, 1], f32, isOutput=False)
    bh = dp("bh", [D, 1], f32, isOutput=False)
    bqr = dp("bqr", [A, 1], f32, isOutput=False)
    balpha = dp("balpha", [P, 1], f32, isOutput=False)
    iota_d = dp("iota", [P, P], f32, isOutput=False)

    off_sub_d = dp("off_sub", [P, CT], i32, isOutput=False)
    off_rel_d = dp("off_rel", [P, CT], i32, isOutput=False)
    off_rix_d = dp("off_rix", [P, CT], i32, isOutput=False)
    obj_f_d = dp("obj_f", [P, CT], f32, isOutput=False)
    ng_off_d = dp("ng_off", [P, T_TILES], i32, isOutput=False)

    out_ht = dp("out_ht", [P, T_TILES * P], f32, isOutput=True)
    out_hnqr = dp("out_hnqr", [T_TILES * P, D], f32, isOutput=True)

    Cmax = max(C_list)

    from contextlib import ExitStack
    with _TC(nc) as tc, ExitStack() as ctx:
        const = ctx.enter_context(tc.tile_pool(name="const", bufs=1))
        meta = ctx.enter_context(tc.tile_pool(name="meta", bufs=1))
        gat = ctx.enter_context(tc.tile_pool(name="gat", bufs=2))
        trn = ctx.enter_context(tc.tile_pool(name="trn", bufs=2))
        ck = ctx.enter_context(tc.tile_pool(name="ck", bufs=3))
        fin = ctx.enter_context(tc.tile_pool(name="fin", bufs=2))
        psg = ctx.enter_context(tc.tile_pool(name="psg", bufs=1, space="PSUM"))
        psc = ctx.enter_context(tc.tile_pool(name="psc", bufs=1, space="PSUM"))
        psa = ctx.enter_context(tc.tile_pool(name="psa", bufs=2, space="PSUM"))
        psf = ctx.enter_context(tc.tile_pool(name="psf", bufs=1, space="PSUM"))

        def load(pool, dram_t, shape, dt, tag):
            t = pool.tile(shape, dt, tag=tag)
            nc.sync.dma_start(t[:], dram_t[:])
            return t

        wz_t_s = load(const, wz_t, [D, D], f16, "wz_t")
        wz_b_s = load(const, wz_b, [D, D], f16, "wz_b")
        uz_s = load(const, uz, [D, D], f16, "uz")
        wr_t_s = load(const, wr_t, [D, D], f16, "wr_t")
        wr_b_s = load(const, wr_b, [D, D], f16, "wr_b")
        ur_s = load(const, ur, [D, D], f16, "ur")
        wh_t_s = load(const, wh_t, [D, D], f16, "wh_t")
        wh_b_s = load(const, wh_b, [D, D], f16, "wh_b")
        uh_s = load(const, uh, [D, D], f16, "uh")
        ws_s = load(const, ws, [D, A], f16, "ws")
        wqr_s = load(const, wqr, [D, A], f16, "wqr")
        walpha_s = load(const, walpha, [A, 1], f16, "walpha")
        whout_s = load(const, wh_out, [D, D], f16, "whout")
        bz_s = load(const, bz, [D, 1], f32, "bz")
        br_s = load(const, br, [D, 1], f32, "br")
        bh_s = load(const, bh, [D, 1], f32, "bh")
        bqr_s = load(const, bqr, [A, 1], f32, "bqr")
        balpha_s = load(const, balpha, [P, 1], f32, "balpha")
        iota_s = load(const, iota_d, [P, P], f32, "iota")
        ones_s = const.tile([P, 1], f16, tag="ones")
        nc.vector.memset(ones_s[:], 1.0)

        off_sub_s = load(meta, off_sub_d, [P, CT], i32, "off_sub")
        off_rel_s = load(meta, off_rel_d, [P, CT], i32, "off_rel")
        off_rix_s = load(meta, off_rix_d, [P, CT], i32, "off_rix")
        obj_f_s = load(meta, obj_f_d, [P, CT], f32, "obj_f")
        ng_off_s = load(meta, ng_off_d, [P, T_TILES], i32, "ng_off")

        mm = nc.tensor.matmul
        act = nc.scalar.activation

        for t in range(T_TILES):
            Ct = C_list[t]
            co = int(col_off[t])
            Et = Ct * P

            hs_raw = gat.tile([P, Cmax * P], f16, tag="hs_raw")
            hr_raw = gat.tile([P, Cmax * P], f16, tag="hr_raw")
            hqr_raw = gat.tile([P, Cmax * P], f16, tag="hqr_raw")
            for c in range(Ct if not ABLATE_NO_GATHER else 1):
                sl = slice(c * P, (c + 1) * P)
                nc.gpsimd.indirect_dma_start(
                    out=hs_raw[:, sl], out_offset=None, in_=hid16[:],
                    in_offset=bass.IndirectOffsetOnAxis(
                        ap=off_sub_s[:, co + c:co + c + 1], axis=0))
                nc.gpsimd.indirect_dma_start(
                    out=hr_raw[:, sl], out_offset=None, in_=rela16[:],
                    in_offset=bass.IndirectOffsetOnAxis(
                        ap=off_rel_s[:, co + c:co + c + 1], axis=0))
                nc.gpsimd.indirect_dma_start(
                    out=hqr_raw[:, sl], out_offset=None, in_=hq16[:],
                    in_offset=bass.IndirectOffsetOnAxis(
                        ap=off_rix_s[:, co + c:co + c + 1], axis=0))

            if ABLATE_GATHER_ONLY:
                continue
            hsT = trn.tile([P, Cmax * P], f16, tag="hsT")
            hrT = trn.tile([P, Cmax * P], f16, tag="hrT")
            hqrT = trn.tile([P, Cmax * P], f16, tag="hqrT")
            nc.sync.dma_start_transpose(
                out=hsT[:, :Et].rearrange("p (k d) -> p k d", k=Ct),
                in_=hs_raw[:, :Et])
            nc.sync.dma_start_transpose(
                out=hrT[:, :Et].rearrange("p (k d) -> p k d", k=Ct),
                in_=hr_raw[:, :Et])
            nc.sync.dma_start_transpose(
                out=hqrT[:, :Et].rearrange("p (k d) -> p k d", k=Ct),
                in_=hqr_raw[:, :Et])

            agg = psa.tile([P, 132], f32, tag="agg")

            for m0 in range(0, Ct, MACRO):
                mc = min(MACRO, Ct - m0)
                Em = mc * P
                sl = slice(m0 * P, m0 * P + Em)

                zp = psg.tile([P, MACRO * P], f32, tag="zp")
                rp = psg.tile([P, MACRO * P], f32, tag="rp")
                hp = psg.tile([P, MACRO * P], f32, tag="hp")
                apre = psg.tile([P, MACRO * P], f32, tag="apre")

                mm(zp[:, :Em], lhsT=wz_t_s[:], rhs=hrT[:, sl],
                   start=True, stop=False)
                mm(zp[:, :Em], lhsT=wz_b_s[:], rhs=hqrT[:, sl],
                   start=False, stop=False)
                mm(zp[:, :Em], lhsT=uz_s[:], rhs=hsT[:, sl],
                   start=False, stop=True)

                mm(rp[:, :Em], lhsT=wr_t_s[:], rhs=hrT[:, sl],
                   start=True, stop=False)
                mm(rp[:, :Em], lhsT=wr_b_s[:], rhs=hqrT[:, sl],
                   start=False, stop=False)
                mm(rp[:, :Em], lhsT=ur_s[:], rhs=hsT[:, sl],
                   start=False, stop=True)

                mm(hp[:, :Em], lhsT=wh_t_s[:], rhs=hrT[:, sl],
                   start=True, stop=False)
                mm(hp[:, :Em], lhsT=wh_b_s[:], rhs=hqrT[:, sl],
                   start=False, stop=False)

                z_sb = ck.tile([P, MACRO * P], f16, tag="z")
                act(z_sb[:, :Em], zp[:, :Em], AF.Sigmoid, bias=bz_s[:, :1])
                r_sb = ck.tile([P, MACRO * P], f16, tag="r")
                act(r_sb[:, :Em], rp[:, :Em], AF.Sigmoid, bias=br_s[:, :1])

                rh = ck.tile([P, MACRO * P], f16, tag="rh")
                nc.vector.tensor_tensor(out=rh[:, :Em], in0=r_sb[:, :Em],
                                        in1=hsT[:, sl], op=ALU.mult)
                mm(hp[:, :Em], lhsT=uh_s[:], rhs=rh[:, :Em],
                   start=False, stop=True)
                ht = ck.tile([P, MACRO * P], f16, tag="ht")
                act(ht[:, :Em], hp[:, :Em], AF.Tanh, bias=bh_s[:, :1])

                # message^T = hsT + z*(ht - hsT)
                dd = ck.tile([P, MACRO * P], f16, tag="dd")
                nc.vector.tensor_tensor(out=dd[:, :Em], in0=ht[:, :Em],
                                        in1=hsT[:, sl], op=ALU.subtract)
                zd = ck.tile([P, MACRO * P], f16, tag="zd")
                nc.vector.tensor_tensor(out=zd[:, :Em], in0=z_sb[:, :Em],
                                        in1=dd[:, :Em], op=ALU.mult)
                msgT = ck.tile([P, MACRO * P], f16, tag="msgT")
                nc.vector.tensor_tensor(out=msgT[:, :Em], in0=zd[:, :Em],
                                        in1=hsT[:, sl], op=ALU.add)

                mm(apre[:, :Em], lhsT=ws_s[:], rhs=msgT[:, :Em],
                   start=True, stop=False)
                mm(apre[:, :Em], lhsT=wqr_s[:], rhs=hqrT[:, sl],
                   start=False, stop=True)
                relu_sb = ck.tile([P, MACRO * P], f16, tag="relu")
                act(relu_sb[:, :Em], apre[:, :Em], AF.Relu, bias=bqr_s[:, :1])

                msgE = ck.tile([P, MACRO * P], f16, tag="msgE")
                nc.sync.dma_start_transpose(
                    out=msgE[:, :Em].rearrange("p (k d) -> p k d", k=mc),
                    in_=msgT[:, :Em])

                acol = psc.tile([P, MACRO], f32, tag="acol")
                for cl in range(mc):
                    c = m0 + cl
                    csl = slice(cl * P, (cl + 1) * P)
                    mm(acol[:, cl:cl + 1], lhsT=relu_sb[:, csl],
                       rhs=walpha_s[:], start=True, stop=True)
                    expc = ck.tile([P, 1], f32, tag="expc")
                    act(expc[:, :1], acol[:, cl:cl + 1], AF.Exp,
                        bias=balpha_s[:, :1])
                    pw = ck.tile([P, P], f16, tag="pw")
                    nc.vector.tensor_scalar(
                        out=pw[:], in0=iota_s[:],
                        scalar1=obj_f_s[:, co + c:co + c + 1],
                        scalar2=expc[:, :1],
                        op0=ALU.is_equal, op1=ALU.mult)
                    # start=True clears the whole PSUM bank, so only the
                    # first matmul of the bank may use it; the sumexp
                    # column relies on that bank clear (a start=False mm
                    # on cleared has_written bits writes fresh).
                    mm(agg[:, 0:P], lhsT=pw[:], rhs=msgE[:, csl],
                       start=(c == 0), stop=(c == Ct - 1),
                       skip_group_check=True)
                    mm(agg[:, P:P + 1], lhsT=pw[:], rhs=ones_s[:],
                       start=False, stop=(c == Ct - 1),
                       skip_group_check=True)

            recip = fin.tile([P, 1], f32, tag="recip")
            nc.vector.reciprocal(recip[:], agg[:, P:P + 1])
            magg = fin.tile([P, P], f16, tag="magg")
            nc.vector.tensor_scalar(out=magg[:], in0=agg[:, 0:P],
                                    scalar1=recip[:, :1], scalar2=None,
                                    op0=ALU.mult)
            maggT = fin.tile([P, P], f16, tag="maggT")
            nc.sync.dma_start_transpose(out=maggT[:], in_=magg[:])
            hf = psf.tile([P, P], f32, tag="hf")
            mm(hf[:], lhsT=whout_s[:], rhs=maggT[:], start=True, stop=True)
            hnew = fin.tile([P, P], f32, tag="hnew")
            act(hnew[:], hf[:], AF.Relu)
            nc.sync.dma_start(out_ht[:, t * P:(t + 1) * P], hnew[:])

            hnq = fin.tile([P, D], f32, tag="hnq")
            nc.gpsimd.indirect_dma_start(
                out=hnq[:], out_offset=None, in_=hq32[:],
                in_offset=bass.IndirectOffsetOnAxis(
                    ap=ng_off_s[:, t:t + 1], axis=0))
            nc.sync.dma_start(out_hnqr[t * P:(t + 1) * P, :], hnq[:])

    return nc


# ----------------------------------------------------------------- kernel()
def kernel(hidden, rela_embed, Wz, Uz, bz, Wr_g, Ur, br, Whh, Uh, bh,
           Ws_attn, Wqr_attn, b_qr, w_alpha, b_alpha, W_h,
           q_rel, edges, n_node):
    _install_wait_splitter()

    hidden = np.asarray(hidden, np.float32)
    rela_embed = np.asarray(rela_embed, np.float32)
    edges = np.asarray(edges)
    q_rel = np.asarray(q_rel)

    meta = _host_prep(hidden, rela_embed, q_rel, edges)
    C_list, col_off, CT = meta["C_list"], meta["col_off"], meta["CT"]

    hq = rela_embed[np.asarray(q_rel, np.int64)]          # [NQ, D] f32

    nc = _build_program(C_list, col_off, CT)

    common = {
        "hid16": hidden.astype(np.float16),
        "rela16": rela_embed.astype(np.float16),
        "hq16": hq.astype(np.float16),
        "hq32": hq.astype(np.float32),
        "wz_t": np.asarray(Wz[:D], np.float16),
        "wz_b": np.asarray(Wz[D:], np.float16),
        "uz": np.asarray(Uz, np.float16),
        "wr_t": np.asarray(Wr_g[:D], np.float16),
        "wr_b": np.asarray(Wr_g[D:], np.float16),
        "ur": np.asarray(Ur, np.float16),
        "wh_t": np.asarray(Whh[:D], np.float16),
        "wh_b": np.asarray(Whh[D:], np.float16),
        "uh": np.asarray(Uh, np.float16),
        "ws": np.asarray(Ws_attn, np.float16),
        "wqr": np.asarray(Wqr_attn, np.float16),
        "walpha": np.asarray(w_alpha, np.float16).reshape(A, 1),
        "wh_out": np.asarray(W_h, np.float16),
        "bz": np.asarray(bz, np.float32).reshape(D, 1),
        "br": np.asarray(br, np.float32).reshape(D, 1),
        "bh": np.asarray(bh, np.float32).reshape(D, 1),
        "bqr": np.asarray(b_qr, np.float32).reshape(A, 1),
        "balpha": np.full((P, 1), float(np.asarray(b_alpha).reshape(-1)[0]),
                          np.float32),
        "iota": np.broadcast_to(np.arange(P, dtype=np.float32),
                                (P, P)).copy(),
    }
    in_maps = []
    for core in range(NCORES):
        m = dict(common)
        m["off_sub"] = meta["off_sub"][core]
        m["off_rel"] = meta["off_rel"][core]
        m["off_rix"] = meta["off_rix"][core]
        m["obj_f"] = meta["obj_f"][core]
        m["ng_off"] = meta["ng_off"][core]
        in_maps.append(m)

    res = run_bass_kernel_spmd(nc, in_maps, list(range(NCORES))).results

    hidden_new = np.empty((N_PAD, D), np.float32)
    h_n_qr = np.empty((N_PAD, D), np.float32)
    for core in range(NCORES):
        lo = core * NODES_PER_CORE
        hi = lo + NODES_PER_CORE
        hidden_new[lo:hi] = res[core]["out_ht"].T
        h_n_qr[lo:hi] = res[core]["out_hnqr"]

    return hidden_new[:N_NODE], h_n_qr[:N_NODE]
